# revision 6
# baseline (speedup 1.0000x reference)
"""Segment-mean pooling (segment_sum / counts) + Linear, on 8 TRN2 NeuronCores.

Strategy: segment-ownership sharding. The host sorts rows by dst_idx and
routes each row to the core that owns its segment range (core i owns
segments [512*i, 512*(i+1))).  Each core computes, fully locally:

    sums_T[h, s] = sum over its rows r with idx[r]==s of x[r, h]

via one-hot matmuls on the TensorEngine (onehot built by a VectorE
is_equal against an iota row), accumulated in PSUM across all row-chunks.
Because rows arrive sorted, chunk c's segments fall in a narrow window
[w_c, w_c + WMAX) shared across cores (w_c = min over cores, WMAX = max
span); the one-hot and the matmul moving operand are only WMAX wide, and
the matmul writes a free-dim slice of the PSUM accumulator.  PSUM
accumulation is opened/closed by rank-1 zero matmuls so the windowed
matmuls are all pure accumulate.

The epilogue applies the Linear (out[s, j] = sums_T[:, s].T @ W.T[h, j],
scaled by 1/(count+eps), + b) and DMAs the core's [512, 256] output
shard; the host concatenates the 8 shards.  No collectives are needed.
Counts are index-only metadata, computed host-side (np.bincount) and
shipped as a per-core [128, 4] reciprocal table.
"""

import numpy as np

import concourse.bass as bass
import concourse.mybir as mybir
from concourse.bass_utils import run_bass_kernel_spmd

N_CORES = 8
S_TOTAL = 4096
S_PER = S_TOTAL // N_CORES  # 512 segments per core
H = 256
EPS = np.float32(1e-8)
PAD_IDX = 9999.0  # sentinel relative idx; never matches iota [0, WMAX)

NXB = 3  # x super-chunk buffers
NOH = 8  # onehot buffers
GSZ = 8  # chunks per DMA group (1024 rows, 512 KB)

_graph_cache: dict = {}


def _build(cap: int, wins: tuple, wmax: int) -> "bass.Bass":
    """Per-core Bass graph. cap % (128*GSZ) == 0; wins[c] = window start."""
    n_chunks = cap // 128
    n_groups = cap // (128 * GSZ)
    assert n_chunks % 2 == 0 and len(wins) == n_chunks
    f16 = mybir.dt.float16
    f32 = mybir.dt.float32

    nc = bass.Bass()

    xz = nc.declare_dram_parameter("xz", [cap, H], f16, isOutput=False)
    idx_d = nc.declare_dram_parameter("idxf", [128, n_chunks], f32, isOutput=False)
    iota_d = nc.declare_dram_parameter("iota", [128, wmax + 512], f16, isOutput=False)
    wt_d = nc.declare_dram_parameter("wt", [H, H], f16, isOutput=False)
    invc_d = nc.declare_dram_parameter("invc", [128, 4], f32, isOutput=False)
    bb_d = nc.declare_dram_parameter("bb", [128, H], f32, isOutput=False)
    out_d = nc.declare_dram_parameter("out", [S_PER, H], f32, isOutput=True)

    # HBM view: row (128*GSZ*g + 128k + p) -> [p, g, k, c]
    xz_v = xz[:, :].rearrange("(g k p) c -> p g k c", p=128, k=GSZ)

    from contextlib import ExitStack

    with ExitStack() as ctx:
        xzb = ctx.enter_context(nc.sbuf_tensor("xzb", [128, NXB, GSZ, H], f16))
        idx_sb = ctx.enter_context(nc.sbuf_tensor("idx_sb", [128, n_chunks], f32))
        oh = ctx.enter_context(nc.sbuf_tensor("oh", [128, NOH, wmax], f16))
        iota_sb = ctx.enter_context(nc.sbuf_tensor("iota_sb", [128, wmax + 512], f16))
        wt_sb = ctx.enter_context(nc.sbuf_tensor("wt_sb", [128, 2, H], f16))
        invc_sb = ctx.enter_context(nc.sbuf_tensor("invc_sb", [128, 4], f32))
        bb_sb = ctx.enter_context(nc.sbuf_tensor("bb_sb", [128, H], f32))
        sums_sb = ctx.enter_context(nc.sbuf_tensor("sums_sb", [128, 2, S_PER], f16))
        out_sb = ctx.enter_context(nc.sbuf_tensor("out_sb", [128, 4, H], f32))
        ps_h0 = ctx.enter_context(nc.psum_tensor("ps_h0", [128, S_PER], f32))
        ps_h1 = ctx.enter_context(nc.psum_tensor("ps_h1", [128, S_PER], f32))
        ps_out = [
            ctx.enter_context(nc.psum_tensor(f"ps_o{t}", [128, H], f32))
            for t in range(4)
        ]
        dma_sem = ctx.enter_context(nc.semaphore("dma_sem"))
        cmp_sem = ctx.enter_context(nc.semaphore("cmp_sem"))
        mm_sem = ctx.enter_context(nc.semaphore("mm_sem"))
        cp_sem = ctx.enter_context(nc.semaphore("cp_sem"))
        mme_sem = ctx.enter_context(nc.semaphore("mme_sem"))
        oe_sem = ctx.enter_context(nc.semaphore("oe_sem"))
        block = ctx.enter_context(nc.Block())

        n_const = 5  # iota, idxf, wt, invc, bb
        ps_h = [ps_h0, ps_h1]
        zlhs = iota_sb[0:1, 0:128]  # junk values; multiplied by zero rhs
        zrhs = iota_sb[0:1, wmax : wmax + 512]  # zeros

        # mm_sem increments: one per chunk pair + one after the closing mms
        def mm_done(chunks):  # sem value meaning "first `chunks` chunks consumed"
            return (chunks + 1) // 2

        @block.sync
        def _(sync):
            sync.dma_start(out=iota_sb[:, :], in_=iota_d[:, :]).then_inc(dma_sem, 16)
            sync.dma_start(out=idx_sb[:, :], in_=idx_d[:, :]).then_inc(dma_sem, 16)
            sync.dma_start(
                out=wt_sb[:, :, :],
                in_=wt_d[:, :].rearrange("(t p) j -> p t j", p=128),
            ).then_inc(dma_sem, 16)
            sync.dma_start(out=invc_sb[:, :], in_=invc_d[:, :]).then_inc(dma_sem, 16)
            sync.dma_start(out=bb_sb[:, :], in_=bb_d[:, :]).then_inc(dma_sem, 16)
            for g in range(n_groups):
                if g >= NXB:
                    sync.wait_ge(mm_sem, mm_done(GSZ * (g - NXB + 1)))
                sync.dma_start(
                    out=xzb[:, g % NXB, :, :], in_=xz_v[:, g, :, :]
                ).then_inc(dma_sem, 16)
            for st in range(4):
                sync.wait_ge(oe_sem, st + 1)
                sync.dma_start(
                    out=out_d[st * 128 : (st + 1) * 128, :], in_=out_sb[:, st, :]
                ).then_inc(dma_sem, 16)
            sync.wait_ge(dma_sem, 16 * (n_const + n_groups + 4))

        @block.vector
        def _(vector):
            for c in range(n_chunks):
                g, k = divmod(c, GSZ)
                if k == 0:
                    vector.wait_ge(dma_sem, 16 * (n_const + g + 1))
                    if c >= NOH:
                        vector.wait_ge(mm_sem, mm_done(c - NOH + 1))
                ins = vector.tensor_scalar(
                    out=oh[:, c % NOH, :],
                    in0=iota_sb[:, 0:wmax],
                    scalar1=idx_sb[:, c : c + 1],
                    scalar2=None,
                    op0=mybir.AluOpType.is_equal,
                )
                if c % 2 == 1:
                    ins.then_inc(cmp_sem, 1)
            # epilogue: PSUM sums -> SBUF (cast to f16 for the W matmul)
            vector.wait_ge(mm_sem, mm_done(n_chunks) + 1)
            vector.tensor_copy(out=sums_sb[:, 0, :], in_=ps_h0[:, :]).then_inc(cp_sem, 1)
            vector.tensor_copy(out=sums_sb[:, 1, :], in_=ps_h1[:, :]).then_inc(cp_sem, 1)
            for st in range(4):
                vector.wait_ge(mme_sem, st + 1)
                vector.tensor_scalar(
                    out=out_sb[:, st, :],
                    in0=ps_out[st][:, :],
                    scalar1=invc_sb[:, st : st + 1],
                    scalar2=None,
                    op0=mybir.AluOpType.mult,
                )
                vector.tensor_tensor(
                    out=out_sb[:, st, :],
                    in0=out_sb[:, st, :],
                    in1=bb_sb[:, :],
                    op=mybir.AluOpType.add,
                ).then_inc(oe_sem, 1)

        @block.tensor
        def _(tensor):
            # open both accumulators (rank-1 zero matmuls over the full banks)
            tensor.wait_ge(dma_sem, 16)
            for hb in range(2):
                tensor.matmul(
                    ps_h[hb][:, :], zlhs, zrhs, start=True, stop=False,
                    skip_group_check=True,
                )
            for c in range(n_chunks):
                g, k = divmod(c, GSZ)
                if c % 2 == 0:
                    tensor.wait_ge(cmp_sem, c // 2 + 1)
                w = wins[c]
                for hb in range(2):
                    ins = tensor.matmul(
                        ps_h[hb][:, w : w + wmax],
                        xzb[:, g % NXB, k, 128 * hb : 128 * (hb + 1)],
                        oh[:, c % NOH, :],
                        start=False,
                        stop=False,
                        skip_group_check=True,
                    )
                if c % 2 == 1:
                    ins.then_inc(mm_sem, 1)
            # close both accumulators
            for hb in range(2):
                ins = tensor.matmul(
                    ps_h[hb][:, :], zlhs, zrhs, start=False, stop=True,
                    skip_group_check=True,
                )
            ins.then_inc(mm_sem, 1)
            # epilogue: out[s, j] = sum_h sums_T[h, s] * wt[h, j]
            tensor.wait_ge(cp_sem, 2)
            for st in range(4):
                tensor.matmul(
                    ps_out[st][:, :],
                    sums_sb[:, 0, st * 128 : (st + 1) * 128],
                    wt_sb[:, 0, :],
                    start=True,
                    stop=False,
                )
                tensor.matmul(
                    ps_out[st][:, :],
                    sums_sb[:, 1, st * 128 : (st + 1) * 128],
                    wt_sb[:, 1, :],
                    start=False,
                    stop=True,
                ).then_inc(mme_sem, 1)

    return nc


def kernel(x, dst_idx, dst_size, W, b):
    x = np.asarray(x)
    idx = np.asarray(dst_idx).astype(np.int64)
    W = np.asarray(W, dtype=np.float32)
    b = np.asarray(b, dtype=np.float32)
    S = int(dst_size)
    assert S == S_TOTAL and x.shape[1] == H

    counts = np.bincount(idx, minlength=S).astype(np.float32)
    inv = np.float32(1.0) / (counts + EPS)  # [4096] f32

    order = np.argsort(idx, kind="stable")
    sidx = idx[order]
    bounds = np.searchsorted(sidx, np.arange(0, S + 1, S_PER))
    percore = bounds[1:] - bounds[:-1]
    round_to = 128 * GSZ
    cap = int(-(-int(percore.max()) // round_to) * round_to)
    n_chunks = cap // 128

    # Per-chunk windows shared across cores: w_c = min over cores of the
    # chunk's lowest local segment; WMAX = max over chunks of span.
    lo = np.full(n_chunks, S_PER - 1, dtype=np.int64)
    hi = np.zeros(n_chunks, dtype=np.int64)
    locs = []
    for i in range(N_CORES):
        li = sidx[bounds[i] : bounds[i + 1]] - S_PER * i
        locs.append(li)
        n_i = len(li)
        nc_i = -(-n_i // 128)
        for c in range(nc_i):
            seg = li[128 * c : 128 * (c + 1)]
            lo[c] = min(lo[c], seg[0])
            hi[c] = max(hi[c], seg[-1])
    hi = np.maximum(hi, lo)
    wins = np.minimum(lo, S_PER - 1)
    span = int((hi - wins).max()) + 1
    wmax = max(16, -(-span // 16) * 16)
    wins = np.minimum(wins, S_PER - wmax)  # keep window in range
    wins_t = tuple(int(v) for v in wins)

    key = (cap, wins_t, wmax)
    nc = _graph_cache.get(key)
    if nc is None:
        nc = _build(cap, wins_t, wmax)
        _graph_cache[key] = nc

    iota_np = np.zeros((128, wmax + 512), dtype=np.float16)
    iota_np[:, :wmax] = np.arange(wmax, dtype=np.float16)
    wt_np = np.ascontiguousarray(W.T).astype(np.float16)
    bb_np = np.ascontiguousarray(np.tile(b, (128, 1)), dtype=np.float32)

    x16 = x.astype(np.float16)
    in_maps = []
    for i in range(N_CORES):
        lo_i, hi_i = bounds[i], bounds[i + 1]
        n_i = hi_i - lo_i
        rows = order[lo_i:hi_i]
        xa = np.zeros((cap, H), dtype=np.float16)
        xa[:n_i] = x16[rows]
        idxf = np.full(cap, PAD_IDX, dtype=np.float32)
        idxf[:n_i] = locs[i] - np.repeat(wins, 128)[:n_i]
        idxf = np.ascontiguousarray(idxf.reshape(n_chunks, 128).T)
        invc_np = np.ascontiguousarray(
            inv[S_PER * i : S_PER * (i + 1)].reshape(4, 128).T
        )
        in_maps.append(
            {
                "xz": xa,
                "idxf": idxf,
                "iota": iota_np,
                "wt": wt_np,
                "invc": invc_np,
                "bb": bb_np,
            }
        )

    res = run_bass_kernel_spmd(nc, in_maps, core_ids=list(range(N_CORES)))
    return np.concatenate([res.results[i]["out"] for i in range(N_CORES)], axis=0)


# revision 9
# speedup vs baseline: 1.0449x; 1.0449x over previous
"""Segment-mean pooling (segment_sum / counts) + Linear, on 8 TRN2 NeuronCores.

Strategy: segment-ownership sharding.  The host sorts rows by dst_idx and
routes each row to the core that owns its segment range (core i owns
segments [512*i, 512*(i+1))), so no collectives are needed; the host
concatenates the 8 output shards.

Per core, the segment sums are computed in [segment, hidden] layout
(segments on PSUM partitions) in two passes:

  Pass 1 (banded): the host packs the first C=16 rows of every segment
  into a dense band of 16-row slots (97% full).  A 128-row chunk then
  covers exactly 8 consecutive segments, and its segment-sum is ONE
  TensorE matmul: stationary = a constant block-ones [128, 32] matrix,
  moving = the x rows [128, 256].  No per-row index handling at all.

  Pass 2 (one-hot tail): rows beyond slot 16 (about 4% of rows) go
  through windowed one-hot matmuls: VectorE builds is_equal one-hots
  against an iota row (precomputed during pass 1), and each chunk's
  matmul writes a narrow 32-aligned window of the accumulators.  Window
  schedule is shared across cores (min/max over cores) so the SPMD graph
  is identical on every core.

Epilogue: scale rows by 1/(count+eps) (host-computed bincount shipped as
a [128, 4] reciprocal table), PE-transpose pooled to [hidden, segment],
apply the Linear as out[s, j] = pooled_T[:, s].T @ W.T[h, j], fuse
bias-add, and DMA the [512, 256] f32 shard.
"""

import numpy as np

import concourse.bass as bass
import concourse.mybir as mybir
from concourse.bass_utils import run_bass_kernel_spmd

N_CORES = 8
S_TOTAL = 4096
S_PER = S_TOTAL // N_CORES  # 512 segments per core
H = 256
EPS = np.float32(1e-8)
PAD_IDX = 9999.0  # sentinel relative idx; never matches iota [0, wmax2)
C = 16  # band capacity (rows per segment in pass 1); must divide 128

NXB = 3  # band super-chunk buffers
GSZ = 16  # chunks per band DMA group (2048 rows, 1 MB)
N_BAND_CHUNKS = S_PER * C // 128  # 64
N_BAND_GROUPS = N_BAND_CHUNKS // GSZ  # 4

_graph_cache: dict = {}


def _build(ov_chunks: int, ov_wins: tuple, wmax2: int) -> "bass.Bass":
    f16 = mybir.dt.float16
    f32 = mybir.dt.float32
    ov_cap = max(ov_chunks, 1) * 128  # xov dram param needs >= 1 chunk

    nc = bass.Bass()

    xb_d = nc.declare_dram_parameter("xb", [S_PER * C, H], f16, isOutput=False)
    xov_d = nc.declare_dram_parameter("xov", [ov_cap, H], f16, isOutput=False)
    ovidx_d = nc.declare_dram_parameter(
        "ovidx", [128, max(ov_chunks, 1)], f32, isOutput=False
    )
    iota_d = nc.declare_dram_parameter("iota", [128, wmax2 + 256], f16, isOutput=False)
    ones_d = nc.declare_dram_parameter("ones32", [128, 4, 32], f16, isOutput=False)
    ident_d = nc.declare_dram_parameter("ident", [128, 128], f16, isOutput=False)
    wt_d = nc.declare_dram_parameter("wt", [H, H], f16, isOutput=False)
    invc_d = nc.declare_dram_parameter("invc", [128, 4], f32, isOutput=False)
    bb_d = nc.declare_dram_parameter("bb", [128, H], f32, isOutput=False)
    out_d = nc.declare_dram_parameter("out", [S_PER, H], f32, isOutput=True)

    xb_v = xb_d[:, :].rearrange("(g k p) c -> p g k c", p=128, k=GSZ)
    xov_v = xov_d[:, :].rearrange("(k p) c -> p k c", p=128)

    from contextlib import ExitStack

    with ExitStack() as ctx:
        xbb = ctx.enter_context(nc.sbuf_tensor("xbb", [128, NXB, GSZ, H], f16))
        xov_sb = ctx.enter_context(
            nc.sbuf_tensor("xov_sb", [128, max(ov_chunks, 1), H], f16)
        )
        oh2 = ctx.enter_context(
            nc.sbuf_tensor("oh2", [128, max(ov_chunks, 1), wmax2], f16)
        )
        ovidx_sb = ctx.enter_context(
            nc.sbuf_tensor("ovidx_sb", [128, max(ov_chunks, 1)], f32)
        )
        iota_sb = ctx.enter_context(nc.sbuf_tensor("iota_sb", [128, wmax2 + 256], f16))
        ones_sb = ctx.enter_context(nc.sbuf_tensor("ones_sb", [128, 4, 32], f16))
        ident_sb = ctx.enter_context(nc.sbuf_tensor("ident_sb", [128, 128], f16))
        wt_sb = ctx.enter_context(nc.sbuf_tensor("wt_sb", [128, 2, H], f16))
        invc_sb = ctx.enter_context(nc.sbuf_tensor("invc_sb", [128, 4], f32))
        bb_sb = ctx.enter_context(nc.sbuf_tensor("bb_sb", [128, H], f32))
        pool_sb = ctx.enter_context(nc.sbuf_tensor("pool_sb", [128, 4, H], f16))
        sums2_sb = ctx.enter_context(nc.sbuf_tensor("sums2_sb", [128, 2, S_PER], f16))
        out_sb = ctx.enter_context(nc.sbuf_tensor("out_sb", [128, 4, H], f32))
        ps_s = [
            ctx.enter_context(nc.psum_tensor(f"ps_s{t}", [128, H], f32))
            for t in range(4)
        ]
        ps_t0 = ctx.enter_context(nc.psum_tensor("ps_t0", [128, S_PER], f16))
        ps_t1 = ctx.enter_context(nc.psum_tensor("ps_t1", [128, S_PER], f16))
        dma_sem = ctx.enter_context(nc.semaphore("dma_sem"))
        dma2_sem = ctx.enter_context(nc.semaphore("dma2_sem"))
        cmp_sem = ctx.enter_context(nc.semaphore("cmp_sem"))
        mm_sem = ctx.enter_context(nc.semaphore("mm_sem"))
        cp_sem = ctx.enter_context(nc.semaphore("cp_sem"))
        mme_sem = ctx.enter_context(nc.semaphore("mme_sem"))
        oe_sem = ctx.enter_context(nc.semaphore("oe_sem"))
        block = ctx.enter_context(nc.Block())

        ps_t = [ps_t0, ps_t1]
        zlhs = iota_sb[0:1, 0:128]  # junk values; multiplied by zero rhs
        zrhs = iota_sb[0:1, wmax2 : wmax2 + 256]  # zeros

        @block.sync
        def _(sync):
            # consts: iota, ovidx, ones32, ident, wt, invc, bb  (7 DMAs)
            sync.dma_start(out=iota_sb[:, :], in_=iota_d[:, :]).then_inc(dma_sem, 16)
            sync.dma_start(out=ovidx_sb[:, :], in_=ovidx_d[:, :]).then_inc(dma_sem, 16)
            sync.dma_start(out=ones_sb[:, :, :], in_=ones_d[:, :, :]).then_inc(
                dma_sem, 16
            )
            sync.dma_start(out=ident_sb[:, :], in_=ident_d[:, :]).then_inc(dma_sem, 16)
            sync.dma_start(
                out=wt_sb[:, :, :],
                in_=wt_d[:, :].rearrange("(t p) j -> p t j", p=128),
            ).then_inc(dma_sem, 16)
            sync.dma_start(out=invc_sb[:, :], in_=invc_d[:, :]).then_inc(dma_sem, 16)
            sync.dma_start(out=bb_sb[:, :], in_=bb_d[:, :]).then_inc(dma_sem, 16)
            # output shards
            for st in range(4):
                sync.wait_ge(oe_sem, st + 1)
                sync.dma_start(
                    out=out_d[st * 128 : (st + 1) * 128, :], in_=out_sb[:, st, :]
                ).then_inc(dma_sem, 16)
            sync.wait_ge(dma_sem, 16 * (7 + 4))

        @block.scalar
        def _(scalar):
            # streamed band groups + overflow rows on the scalar HWDGE queue
            for g in range(N_BAND_GROUPS):
                if g >= NXB:
                    scalar.wait_ge(mm_sem, g - NXB + 1)
                scalar.dma_start(
                    out=xbb[:, g % NXB, :, :], in_=xb_v[:, g, :, :]
                ).then_inc(dma2_sem, 16)
            scalar.dma_start(out=xov_sb[:, :, :], in_=xov_v[:, :, :]).then_inc(
                dma2_sem, 16
            )
            scalar.wait_ge(dma2_sem, 16 * (N_BAND_GROUPS + 1))

        @block.vector
        def _(vector):
            # pass-2 one-hots, precomputed while PE runs the band pass
            if ov_chunks:
                vector.wait_ge(dma_sem, 32)  # iota + ovidx
                for oc in range(ov_chunks):
                    ins = vector.tensor_scalar(
                        out=oh2[:, oc, :],
                        in0=iota_sb[:, 0:wmax2],
                        scalar1=ovidx_sb[:, oc : oc + 1],
                        scalar2=None,
                        op0=mybir.AluOpType.is_equal,
                    )
                ins.then_inc(cmp_sem, 1)
            # epilogue
            vector.wait_ge(mm_sem, N_BAND_GROUPS + 1)
            for t in range(4):
                ins = vector.tensor_copy(out=pool_sb[:, t, :], in_=ps_s[t][:, :])
            ins.then_inc(cp_sem, 1)
            vector.wait_ge(mme_sem, 1)  # transposes done
            vector.tensor_copy(out=sums2_sb[:, 0, :], in_=ps_t0[:, :])
            vector.tensor_copy(out=sums2_sb[:, 1, :], in_=ps_t1[:, :]).then_inc(
                cp_sem, 1
            )
            for st in range(4):
                vector.wait_ge(mme_sem, 2 + st)
                vector.scalar_tensor_tensor(
                    out=out_sb[:, st, :],
                    in0=ps_s[st][:, :],
                    scalar=invc_sb[:, st : st + 1],
                    in1=bb_sb[:, :],
                    op0=mybir.AluOpType.mult,
                    op1=mybir.AluOpType.add,
                ).then_inc(oe_sem, 1)

        @block.tensor
        def _(tensor):
            tensor.wait_ge(dma_sem, 16 * 7)  # all consts (ones32, iota zeros)
            # pass 1: banded segment sums; chunk c covers segs [8c, 8c+8)
            for c in range(N_BAND_CHUNKS):
                g, k = divmod(c, GSZ)
                if k == 0:
                    tensor.wait_ge(dma2_sem, 16 * (g + 1))
                b, v = divmod(c, 4)
                t, poff = b // 4, 32 * (b % 4)
                ins = tensor.matmul(
                    ps_s[t][poff : poff + 32, :],
                    ones_sb[:, v, :],
                    xbb[:, g % NXB, k, :],
                    start=(v == 0),
                    stop=False,
                    skip_group_check=True,
                    tile_position=(0, poff),
                )
                if k == GSZ - 1:
                    ins.then_inc(mm_sem, 1)
            # pass 2: one-hot tail
            if ov_chunks:
                tensor.wait_ge(dma2_sem, 16 * (N_BAND_GROUPS + 1))
                tensor.wait_ge(cmp_sem, 1)
                for oc in range(ov_chunks):
                    w = ov_wins[oc]
                    for p in range(wmax2 // 32):
                        seg0 = w + 32 * p
                        t, poff = seg0 // 128, seg0 % 128
                        tensor.matmul(
                            ps_s[t][poff : poff + 32, :],
                            oh2[:, oc, 32 * p : 32 * (p + 1)],
                            xov_sb[:, oc, :],
                            start=False,
                            stop=False,
                            skip_group_check=True,
                            tile_position=(0, poff),
                        )
            # close the accumulators
            for t in range(4):
                ins = tensor.matmul(
                    ps_s[t][:, :], zlhs, zrhs, start=False, stop=True,
                    skip_group_check=True,
                )
            ins.then_inc(mm_sem, 1)
            # transposes: pooled [s, h] -> pooled_T [h, s]
            tensor.wait_ge(cp_sem, 1)
            for t in range(4):
                for hb in range(2):
                    ins = tensor.transpose(
                        ps_t[hb][:, 128 * t : 128 * (t + 1)],
                        pool_sb[:, t, 128 * hb : 128 * (hb + 1)],
                        ident_sb[:, :],
                    )
            ins.then_inc(mme_sem, 1)
            # Linear: out[s, j] = sum_h pooled_T[h, s] * wt[h, j]
            tensor.wait_ge(cp_sem, 2)
            for st in range(4):
                tensor.matmul(
                    ps_s[st][:, :],
                    sums2_sb[:, 0, st * 128 : (st + 1) * 128],
                    wt_sb[:, 0, :],
                    start=True,
                    stop=False,
                )
                tensor.matmul(
                    ps_s[st][:, :],
                    sums2_sb[:, 1, st * 128 : (st + 1) * 128],
                    wt_sb[:, 1, :],
                    start=False,
                    stop=True,
                ).then_inc(mme_sem, 1)

    return nc


def kernel(x, dst_idx, dst_size, W, b):
    x = np.asarray(x)
    idx = np.asarray(dst_idx).astype(np.int64)
    W = np.asarray(W, dtype=np.float32)
    b = np.asarray(b, dtype=np.float32)
    S = int(dst_size)
    assert S == S_TOTAL and x.shape[1] == H

    counts = np.bincount(idx, minlength=S).astype(np.float32)
    inv = np.float32(1.0) / (counts + EPS)  # [4096] f32

    order = np.argsort(idx, kind="stable")
    sidx = idx[order]
    bounds = np.searchsorted(sidx, np.arange(0, S + 1, S_PER))

    x16 = x.astype(np.float16)

    # split each core's rows into band (rank < C) and overflow (rank >= C)
    bands, ovs, ovsegs = [], [], []
    for i in range(N_CORES):
        lo_i, hi_i = bounds[i], bounds[i + 1]
        n_i = hi_i - lo_i
        li = (sidx[lo_i:hi_i] - S_PER * i).astype(np.int64)
        rows = order[lo_i:hi_i]
        starts = np.searchsorted(li, np.arange(S_PER + 1))
        rank = np.arange(n_i) - starts[li]
        bm = rank < C
        xband = np.zeros((S_PER * C, H), dtype=np.float16)
        xband[li[bm] * C + rank[bm]] = x16[rows[bm]]
        bands.append(xband)
        ovs.append(x16[rows[~bm]])
        ovsegs.append(li[~bm])

    ov_chunks = max(-(-len(s) // 128) for s in ovsegs)
    ov_cap = max(ov_chunks, 1) * 128

    # shared overflow window schedule (32-aligned starts)
    wins = []
    spans = []
    for oc in range(ov_chunks):
        lo_w, hi_w = S_PER - 1, 0
        for s in ovsegs:
            seg = s[128 * oc : 128 * (oc + 1)]
            if len(seg):
                lo_w = min(lo_w, int(seg[0]))
                hi_w = max(hi_w, int(seg[-1]))
        hi_w = max(hi_w, lo_w)
        w = (lo_w // 32) * 32
        wins.append(w)
        spans.append(hi_w - w + 1)
    wmax2 = max(32, -(-max(spans, default=1) // 32) * 32) if ov_chunks else 32
    wins = [min(w, S_PER - wmax2) for w in wins]
    wins_t = tuple(wins)

    key = (ov_chunks, wins_t, wmax2)
    nc = _graph_cache.get(key)
    if nc is None:
        nc = _build(ov_chunks, wins_t, wmax2)
        _graph_cache[key] = nc

    iota_np = np.zeros((128, wmax2 + 256), dtype=np.float16)
    iota_np[:, :wmax2] = np.arange(wmax2, dtype=np.float16)
    ones_np = np.zeros((128, 4, 32), dtype=np.float16)
    r = np.arange(128)
    for v in range(4):
        ones_np[r, v, 8 * v + r // C] = 1.0
    ident_np = np.eye(128, dtype=np.float16)
    wt_np = np.ascontiguousarray(W.T).astype(np.float16)
    bb_np = np.ascontiguousarray(np.tile(b, (128, 1)), dtype=np.float32)

    in_maps = []
    for i in range(N_CORES):
        n_ov = len(ovsegs[i])
        xov = np.zeros((ov_cap, H), dtype=np.float16)
        xov[:n_ov] = ovs[i]
        ovidx = np.full(ov_cap, PAD_IDX, dtype=np.float32)
        if ov_chunks:
            ovidx[:n_ov] = ovsegs[i] - np.repeat(wins, 128)[:n_ov]
        ovidx = np.ascontiguousarray(
            ovidx.reshape(max(ov_chunks, 1), 128).T
        ).astype(np.float32)
        invc_np = np.ascontiguousarray(
            inv[S_PER * i : S_PER * (i + 1)].reshape(4, 128).T
        )
        in_maps.append(
            {
                "xb": bands[i],
                "xov": xov,
                "ovidx": ovidx,
                "iota": iota_np,
                "ones32": ones_np,
                "ident": ident_np,
                "wt": wt_np,
                "invc": invc_np,
                "bb": bb_np,
            }
        )

    res = run_bass_kernel_spmd(nc, in_maps, core_ids=list(range(N_CORES)))
    return np.concatenate([res.results[i]["out"] for i in range(N_CORES)], axis=0)
